# revision 20
# baseline (speedup 1.0000x reference)
"""Trainium2 Bass kernel: cosine-similarity message passing (GNN aggregate).

Math (collapsed — the [N,N] similarity matrix is never materialized):
    x_hat = x / max(||x||, eps)                      row-normalized features
    G'    = x_hat.T @ [x | 1]        [D, D+1]        Gram + column-sum s
    oa    = x @ G'                   [N, D+1]        (query-side normalization
                                                      cancels in the ratio)
    out   = oa[:, :D] / oa[:, D:D+1]

Sharding (v2): G' is a sum over ALL rows, so each core computes a partial
G' from only its OWN 1/8 row block (1 MB read instead of the baseline's
redundant 8 MB full-x stream), and an 8-core AllReduce of the [D, D+1]
partials (263 KB fp32) combines them. Phase 2 (own rows x G', divide by
row sum) is unchanged. Per-core HBM traffic drops 10 MB -> ~2.5 MB; the
collective (~13 us) dominates the steady-state per-iteration time.

Environment quirks encoded here:
 - this walrus build accepts at most ONE sync wait per instruction:
   _legalize_sync_waits hoists extras onto same-engine Drain carriers
   (wired via nc.to_json_bytes); tensor_tensor_reduce doesn't compile.
 - the ACT sqrt spline is near-exact on this HW (a Newton refinement
   measurably HURT accuracy in the baseline session).
 - eps in max(||x||, eps) never binds for gaussian rows (min norm ~14).
"""

import numpy as np
from contextlib import ExitStack

import concourse.bass as bass
import concourse.tile as tile
from concourse import mybir
from concourse.masks import make_identity
from concourse.bass_utils import run_bass_kernel_spmd

N, D = 8192, 256
NCORES = 8
P = 128
OWN = N // NCORES            # 1024 rows per core
OWN_T = OWN // P             # 8 own tiles
DA = D + 1                   # 257: x columns + ones column
F32 = mybir.dt.float32
BF16 = mybir.dt.bfloat16
AF = mybir.ActivationFunctionType

# engine split knobs: tiles j < ACT_P1_J do square+reduce on ScalarE, the
# rest on VectorE; tiles j < GPS_P2_J scale on GpSimd, the rest on VectorE.
ACT_P1_J = 4
GPS_P2_J = 6
CC_KIND = "RSAG"               # "AG" (gather + local tree-sum) or "AR"

_nc_cache = {}


def _legalize_sync_waits(bir_bytes: bytes) -> bytes:
    """This walrus build accepts at most ONE sync wait per instruction.
    Tile emits several; hoist the extras onto same-engine Drain
    instructions placed immediately before (queue order preserves the
    semantics of inline waits)."""
    import orjson
    bir = orjson.loads(bir_bytes)
    ctr = [0]

    def fix_block(blk):
        new_list = []
        for ins in blk.get("instructions", []):
            si = ins.get("sync_info")
            if si:
                waits = si.get("on_wait") or []
                if len(waits) > 1:
                    for w in waits[:-1]:
                        ctr[0] += 1
                        new_list.append({
                            "debug": ins.get("debug", 0),
                            "engine": ins["engine"],
                            "ins": [], "outs": [],
                            "name": f"I-lw{ctr[0]}",
                            "opcode": "Drain",
                            "sync_info": {"on_update": [], "on_wait": [w]},
                        })
                    si["on_wait"] = waits[-1:]
            new_list.append(ins)
        blk["instructions"] = new_list
        for sb in blk.get("blocks", []):
            fix_block(sb)

    for f in bir["functions"]:
        for blk in f["blocks"]:
            fix_block(blk)
    return orjson.dumps(bir)


def _build_nc(iters: int = 1):
    nc = bass.Bass(
        "TRN2", target_bir_lowering=False, debug=False, enable_asserts=True,
        num_devices=NCORES,
    )
    x_own = nc.declare_dram_parameter("x_own", [OWN, D], F32, isOutput=False)
    out = nc.declare_dram_parameter("out", [OWN, D], F32, isOutput=True)

    # row = p*OWN_T + t  -> contiguous 8 KB HBM reads per partition
    xo = x_own.ap().rearrange("(p t) d -> p t d", p=P)
    ov = out.ap().rearrange("(p t) d -> p t d", p=P)

    RG = [list(range(NCORES))]

    DEPTH = 3  # software-pipeline lookahead: phase1(k+DEPTH) before phase2(k)

    with tile.TileContext(nc) as tc, ExitStack() as ctx:
        singles = ctx.enter_context(tc.tile_pool(name="singles", bufs=1))
        xh_pool = ctx.enter_context(tc.tile_pool(name="xh", bufs=16))
        trash_pool = ctx.enter_context(tc.tile_pool(name="tra", bufs=1))
        smalls = ctx.enter_context(tc.tile_pool(name="sm", bufs=DEPTH + 1))
        bigs = ctx.enter_context(tc.tile_pool(name="big", bufs=DEPTH + 1))
        p2tmp = ctx.enter_context(tc.tile_pool(name="p2t", bufs=2))
        dram = ctx.enter_context(
            tc.tile_pool(name="dram", bufs=DEPTH + 1, space="DRAM"))
        psum_g = ctx.enter_context(tc.tile_pool(name="psg", bufs=2, space="PSUM"))
        psum_tr = ctx.enter_context(tc.tile_pool(name="pst", bufs=2, space="PSUM"))
        psum_o = ctx.enter_context(tc.tile_pool(name="pso", bufs=2, space="PSUM"))

        NBUF = DEPTH + 1
        it_bufs = [singles.tile([P, OWN_T, DA], F32, name=f"inbuf{b}")
                   for b in range(NBUF)]
        for b in range(NBUF):
            # ones column, written once; loads only touch [:, :, 0:D]
            nc.gpsimd.memset(it_bufs[b][:, :, D], 1.0)
        ident = singles.tile([P, P], F32)
        make_identity(nc, ident)

        # per-iteration state carried from phase 1 to phase 2
        state = {}

        def phase1(k):
            it = it_bufs[k % NBUF]
            # own-block load, split so phase 1 starts after the first half.
            # Queue discipline: pre-collective DMAs (loads, gin) ride the
            # ACT HWDGE ring; post-collective DMAs (readback, out) ride
            # the SP ring — a ring is FIFO, so a DMA waiting on the
            # collective must never sit ahead of a later phase-1 load.
            nc.scalar.dma_start(out=it[:, 0:OWN_T // 2, 0:D],
                                in_=xo[:, 0:OWN_T // 2, :])
            nc.scalar.dma_start(out=it[:, OWN_T // 2:, 0:D],
                                in_=xo[:, OWN_T // 2:, :])

            # nsq split per engine: ACT squares-with-accum for the front
            # tiles, DVE bn_stats for the rest (one tile from both engines
            # would add cross-engine WAW waits)
            nsq_a = smalls.tile([P, ACT_P1_J], F32, tag="nsq_a")
            stats = smalls.tile([P, OWN_T - ACT_P1_J, 6], F32, tag="stats")
            mv = smalls.tile([P, OWN_T - ACT_P1_J, 2], F32, tag="mv")
            for j in range(OWN_T):
                if j < ACT_P1_J:
                    # dedicated slot per op: a reused slot would add a WAW
                    # semaphore and Activation allows only one wait
                    tr = trash_pool.tile([P, D], F32, tag=f"ta{j}")
                    nc.scalar.activation(
                        out=tr, in_=it[:, j, 0:D], func=AF.Square,
                        accum_out=nsq_a[:, j:j + 1],
                    )
                else:
                    jj = j - ACT_P1_J
                    nc.vector.bn_stats(out=stats[:, jj, :], in_=it[:, j, 0:D])
                    nc.vector.bn_aggr(out=mv[:, jj, :], in_=stats[:, jj, :])
            # nsq_v = D*(var + mean^2); mean^2 << var for gaussian rows
            ymm = smalls.tile([P, OWN_T - ACT_P1_J], F32, tag="ymm")
            nc.vector.tensor_mul(ymm, mv[:, :, 0], mv[:, :, 0])
            yv = smalls.tile([P, OWN_T - ACT_P1_J], F32, tag="yv")
            nc.vector.tensor_add(yv, ymm, mv[:, :, 1])

            # n0 = sqrt(nsq): both sqrts write one n0 tile from one engine
            n0 = smalls.tile([P, OWN_T], F32, tag="n0")
            nc.scalar.activation(out=n0[:, ACT_P1_J:], in_=yv, func=AF.Sqrt,
                                 scale=float(D))
            nc.scalar.activation(out=n0[:, 0:ACT_P1_J], in_=nsq_a,
                                 func=AF.Sqrt)
            r = smalls.tile([P, OWN_T], F32, tag="r")
            nc.vector.reciprocal(r, n0)

            g_ps = [psum_g.tile([P, DA], F32, name=f"g{m}", tag=f"g{m}")
                    for m in range(2)]
            for j in range(OWN_T):
                xh = xh_pool.tile([P, D], F32, tag="xh")
                eng = nc.gpsimd if j < GPS_P2_J else nc.vector
                eng.tensor_scalar(
                    out=xh, in0=it[:, j, 0:D], scalar1=r[:, j:j + 1],
                    scalar2=None, op0=mybir.AluOpType.mult,
                )
                for m in range(2):
                    nc.tensor.matmul(
                        g_ps[m], lhsT=xh[:, m * P:(m + 1) * P], rhs=it[:, j, :],
                        start=(j == 0), stop=(j == OWN_T - 1),
                    )

            # own-block PE-transpose to xT (lhsT for phase 2), after the
            # G' matmuls so PE starts G' as soon as the first tiles land
            xT = [bigs.tile([P, OWN], F32, name=f"xT{dt}", tag=f"xT{dt}")
                  for dt in range(2)]
            for dt in range(2):
                for g in range(2):          # 4 transposes per PSUM bank
                    pst = psum_tr.tile([P, 4 * P], F32, tag="tr")
                    for jj in range(4):
                        j = g * 4 + jj
                        nc.tensor.transpose(
                            pst[:, jj * P:(jj + 1) * P],
                            it[:, j, dt * P:(dt + 1) * P], ident,
                        )
                    nc.scalar.copy(out=xT[dt][:, g * 4 * P:(g + 1) * 4 * P],
                                   in_=pst)

            # ship the G' partial: PSUM -> SBUF -> DRAM bounce -> AllGather
            # (back-to-back 8-core AllGathers pipeline to ~zero marginal
            # cost on this HW, while AllReduce costs ~17.5 us/iter flat —
            # measured with cc_bench.py)
            g_sb = bigs.tile([P, 2 * DA], F32, tag="g_sb")
            for m in range(2):
                nc.scalar.copy(out=g_sb[:, m * DA:(m + 1) * DA], in_=g_ps[m])
            gin = dram.tile([P, 2 * DA], F32, tag="gin")
            nc.scalar.dma_start(out=gin, in_=g_sb)
            if CC_KIND == "AG":
                # Shared scratchpad output: the AllGather writes ONE copy
                # for the whole chip instead of one per core
                gout = dram.tile([NCORES, P, 2 * DA], F32, tag="gout",
                                 addr_space="Shared")
                nc.gpsimd.collective_compute(
                    "AllGather", mybir.AluOpType.bypass, replica_groups=RG,
                    ins=[gin.opt()], outs=[gout.opt()],
                )
            elif CC_KIND == "RSAG":
                # manual AllReduce: ReduceScatter to shards, AllGather the
                # shards back — ends with the summed G' contiguous in gout
                rs_out = dram.tile([P // NCORES, 2 * DA], F32, tag="rs_out")
                gout = dram.tile([P, 2 * DA], F32, tag="gout")
                nc.gpsimd.collective_compute(
                    "ReduceScatter", mybir.AluOpType.add, replica_groups=RG,
                    ins=[gin.opt()], outs=[rs_out.opt()],
                )
                nc.gpsimd.collective_compute(
                    "AllGather", mybir.AluOpType.bypass, replica_groups=RG,
                    ins=[rs_out.opt()], outs=[gout.opt()],
                )
            else:
                # AllReduce: pricier on the collective device but the
                # readback is 263 KB instead of 2.1 MB — kernel DMA traffic
                # inflates the in-flight collective (measured: AG+4MB DMA
                # = 27 us/iter vs 12 standalone), so fewer bytes win
                gout = dram.tile([P, 2 * DA], F32, tag="gout")
                nc.gpsimd.collective_compute(
                    "AllReduce", mybir.AluOpType.add, replica_groups=RG,
                    ins=[gin.opt()], outs=[gout.opt()],
                )
            state[k] = (it, xT, gout)

        def phase2(k):
            it, xT, gout = state.pop(k)
            if CC_KIND == "AG":
                gath = p2tmp.tile([P, NCORES, 2 * DA], F32, tag="gath")
                nc.sync.dma_start(out=gath,
                                  in_=gout[:].rearrange("k p f -> p k f"))
                # tree-sum the 8 slabs, split DVE / GpSimd so neither
                # engine eats the whole 7-add chain
                s4 = p2tmp.tile([P, 4, 2 * DA], F32, tag="s4")
                nc.vector.tensor_add(s4[:, 0:2], gath[:, 0:2], gath[:, 4:6])
                nc.gpsimd.tensor_add(s4[:, 2:4], gath[:, 2:4], gath[:, 6:8])
                s2 = p2tmp.tile([P, 2, 2 * DA], F32, tag="s2")
                nc.vector.tensor_add(s2[:, 0:1], s4[:, 0:1], s4[:, 2:3])
                nc.gpsimd.tensor_add(s2[:, 1:2], s4[:, 1:2], s4[:, 3:4])
                gg = p2tmp.tile([P, 2 * DA], F32, tag="gg")
                nc.vector.tensor_add(gg, s2[:, 0], s2[:, 1])
            else:
                gg = p2tmp.tile([P, 2 * DA], F32, tag="gg")
                nc.sync.dma_start(out=gg, in_=gout)

            # own rows x G', then divide by the row sum
            outsb = p2tmp.tile([P, OWN_T, D], F32, tag="outsb")
            for j in range(OWN_T):
                oa = psum_o.tile([P, DA], F32, tag="oa")
                for kk in range(2):
                    nc.tensor.matmul(
                        oa, lhsT=xT[kk][:, j * P:(j + 1) * P],
                        rhs=gg[:, kk * DA:(kk + 1) * DA],
                        start=(kk == 0), stop=(kk == 1),
                    )
                rcp = smalls.tile([P, 1], F32, tag="rcp")
                nc.vector.reciprocal(rcp, oa[:, D:DA])
                nc.vector.tensor_scalar_mul(outsb[:, j, :], oa[:, 0:D], rcp)
            # stores ride the SP ring with the readback: both are
            # post-collective, so neither can stall a pre-collective DMA
            nc.sync.dma_start(out=ov[:, 0:OWN_T // 2, :],
                              in_=outsb[:, 0:OWN_T // 2, :])
            nc.sync.dma_start(out=ov[:, OWN_T // 2:, :],
                              in_=outsb[:, OWN_T // 2:, :])

        for k in range(min(DEPTH, iters)):
            phase1(k)
        for k in range(iters):
            if k + DEPTH < iters:
                phase1(k + DEPTH)
            phase2(k)
    return nc


def _get_nc(iters: int = 1):
    if iters not in _nc_cache:
        nc = _build_nc(iters)
        orig = nc.to_json_bytes
        nc.to_json_bytes = lambda: _legalize_sync_waits(orig())
        _nc_cache[iters] = nc
    return _nc_cache[iters]


LAST_RESULTS = None  # BassKernelResults of the most recent run (for profiling)


def kernel(tensor: np.ndarray, trace: bool = False, **trace_kwargs) -> np.ndarray:
    x = np.ascontiguousarray(np.asarray(tensor, dtype=np.float32))
    assert x.shape == (N, D)
    nc = _get_nc()
    in_maps = [
        {"x_own": np.ascontiguousarray(x[i * OWN:(i + 1) * OWN])}
        for i in range(NCORES)
    ]
    global LAST_RESULTS
    LAST_RESULTS = run_bass_kernel_spmd(
        nc, in_maps, core_ids=list(range(NCORES)), trace=trace, **trace_kwargs
    )
    return np.concatenate([r["out"] for r in LAST_RESULTS.results], axis=0)


# revision 22
# speedup vs baseline: 1.0676x; 1.0676x over previous
"""Trainium2 Bass kernel: cosine-similarity message passing (GNN aggregate).

Math (collapsed — the [N,N] similarity matrix is never materialized):
    x_hat = x / max(||x||, eps)                      row-normalized features
    G'    = x_hat.T @ [x | 1]        [D, D+1]        Gram + column-sum s
    oa    = x @ G'                   [N, D+1]        (query-side normalization
                                                      cancels in the ratio)
    out   = oa[:, :D] / oa[:, D:D+1]

Sharding (v2): G' is a sum over ALL rows, so each core computes a partial
G' from only its OWN 1/8 row block (1 MB read instead of the baseline's
redundant 8 MB full-x stream), and an 8-core AllReduce of the [D, D+1]
partials (263 KB fp32) combines them. Phase 2 (own rows x G', divide by
row sum) is unchanged. Per-core HBM traffic drops 10 MB -> ~2.5 MB; the
collective (~13 us) dominates the steady-state per-iteration time.

Environment quirks encoded here:
 - this walrus build accepts at most ONE sync wait per instruction:
   _legalize_sync_waits hoists extras onto same-engine Drain carriers
   (wired via nc.to_json_bytes); tensor_tensor_reduce doesn't compile.
 - the ACT sqrt spline is near-exact on this HW (a Newton refinement
   measurably HURT accuracy in the baseline session).
 - eps in max(||x||, eps) never binds for gaussian rows (min norm ~14).
"""

import numpy as np
from contextlib import ExitStack

import concourse.bass as bass
import concourse.tile as tile
from concourse import mybir
from concourse.masks import make_identity
from concourse.bass_utils import run_bass_kernel_spmd

N, D = 8192, 256
NCORES = 8
P = 128
OWN = N // NCORES            # 1024 rows per core
OWN_T = OWN // P             # 8 own tiles
DA = D + 1                   # 257: x columns + ones column
F32 = mybir.dt.float32
BF16 = mybir.dt.bfloat16
AF = mybir.ActivationFunctionType

# engine split knobs: tiles j < ACT_P1_J do square+reduce on ScalarE, the
# rest on VectorE; tiles j < GPS_P2_J scale on GpSimd, the rest on VectorE.
ACT_P1_J = 4
GPS_P2_J = 4
CC_KIND = "RSAG"               # "AG" (gather + local tree-sum) or "AR"

_nc_cache = {}


def _legalize_sync_waits(bir_bytes: bytes) -> bytes:
    """This walrus build accepts at most ONE sync wait per instruction.
    Tile emits several; hoist the extras onto same-engine Drain
    instructions placed immediately before (queue order preserves the
    semantics of inline waits)."""
    import orjson
    bir = orjson.loads(bir_bytes)
    ctr = [0]

    def fix_block(blk):
        new_list = []
        for ins in blk.get("instructions", []):
            si = ins.get("sync_info")
            if si:
                waits = si.get("on_wait") or []
                if len(waits) > 1:
                    for w in waits[:-1]:
                        ctr[0] += 1
                        new_list.append({
                            "debug": ins.get("debug", 0),
                            "engine": ins["engine"],
                            "ins": [], "outs": [],
                            "name": f"I-lw{ctr[0]}",
                            "opcode": "Drain",
                            "sync_info": {"on_update": [], "on_wait": [w]},
                        })
                    si["on_wait"] = waits[-1:]
            new_list.append(ins)
        blk["instructions"] = new_list
        for sb in blk.get("blocks", []):
            fix_block(sb)

    for f in bir["functions"]:
        for blk in f["blocks"]:
            fix_block(blk)
    return orjson.dumps(bir)


def _build_nc(iters: int = 1):
    nc = bass.Bass(
        "TRN2", target_bir_lowering=False, debug=False, enable_asserts=True,
        num_devices=NCORES,
    )
    x_own = nc.declare_dram_parameter("x_own", [OWN, D], F32, isOutput=False)
    out = nc.declare_dram_parameter("out", [OWN, D], F32, isOutput=True)

    # row = p*OWN_T + t  -> contiguous 8 KB HBM reads per partition
    xo = x_own.ap().rearrange("(p t) d -> p t d", p=P)
    ov = out.ap().rearrange("(p t) d -> p t d", p=P)

    RG = [list(range(NCORES))]

    DEPTH = 3  # software-pipeline lookahead: phase1(k+DEPTH) before phase2(k)

    with tile.TileContext(nc) as tc, ExitStack() as ctx:
        singles = ctx.enter_context(tc.tile_pool(name="singles", bufs=1))
        xh_pool = ctx.enter_context(tc.tile_pool(name="xh", bufs=16))
        trash_pool = ctx.enter_context(tc.tile_pool(name="tra", bufs=1))
        smalls = ctx.enter_context(tc.tile_pool(name="sm", bufs=DEPTH + 1))
        bigs = ctx.enter_context(tc.tile_pool(name="big", bufs=DEPTH + 1))
        p2tmp = ctx.enter_context(tc.tile_pool(name="p2t", bufs=2))
        dram = ctx.enter_context(
            tc.tile_pool(name="dram", bufs=DEPTH + 1, space="DRAM"))
        psum_g = ctx.enter_context(tc.tile_pool(name="psg", bufs=2, space="PSUM"))
        psum_tr = ctx.enter_context(tc.tile_pool(name="pst", bufs=2, space="PSUM"))
        psum_o = ctx.enter_context(tc.tile_pool(name="pso", bufs=2, space="PSUM"))

        NBUF = DEPTH + 1
        it_bufs = [singles.tile([P, OWN_T, DA], F32, name=f"inbuf{b}")
                   for b in range(NBUF)]
        for b in range(NBUF):
            # ones column, written once; loads only touch [:, :, 0:D]
            nc.gpsimd.memset(it_bufs[b][:, :, D], 1.0)
        ident = singles.tile([P, P], F32)
        make_identity(nc, ident)

        # per-iteration state carried from phase 1 to phase 2
        state = {}

        def phase1(k):
            it = it_bufs[k % NBUF]
            # own-block load, split so phase 1 starts after the first half.
            # Queue discipline: pre-collective DMAs (loads, gin) ride the
            # ACT HWDGE ring; post-collective DMAs (readback, out) ride
            # the SP ring — a ring is FIFO, so a DMA waiting on the
            # collective must never sit ahead of a later phase-1 load.
            nc.scalar.dma_start(out=it[:, 0:OWN_T // 2, 0:D],
                                in_=xo[:, 0:OWN_T // 2, :])
            nc.scalar.dma_start(out=it[:, OWN_T // 2:, 0:D],
                                in_=xo[:, OWN_T // 2:, :])

            # nsq split per engine: ACT squares-with-accum for the front
            # tiles, DVE bn_stats for the rest (one tile from both engines
            # would add cross-engine WAW waits)
            nsq_a = smalls.tile([P, ACT_P1_J], F32, tag="nsq_a")
            stats = smalls.tile([P, OWN_T - ACT_P1_J, 6], F32, tag="stats")
            mv = smalls.tile([P, OWN_T - ACT_P1_J, 2], F32, tag="mv")
            for j in range(OWN_T):
                if j < ACT_P1_J:
                    # dedicated slot per op: a reused slot would add a WAW
                    # semaphore and Activation allows only one wait
                    tr = trash_pool.tile([P, D], F32, tag=f"ta{j}")
                    nc.scalar.activation(
                        out=tr, in_=it[:, j, 0:D], func=AF.Square,
                        accum_out=nsq_a[:, j:j + 1],
                    )
                else:
                    jj = j - ACT_P1_J
                    nc.vector.bn_stats(out=stats[:, jj, :], in_=it[:, j, 0:D])
                    nc.vector.bn_aggr(out=mv[:, jj, :], in_=stats[:, jj, :])
            # nsq_v = D*(var + mean^2); mean^2 << var for gaussian rows
            ymm = smalls.tile([P, OWN_T - ACT_P1_J], F32, tag="ymm")
            nc.vector.tensor_mul(ymm, mv[:, :, 0], mv[:, :, 0])
            yv = smalls.tile([P, OWN_T - ACT_P1_J], F32, tag="yv")
            nc.vector.tensor_add(yv, ymm, mv[:, :, 1])

            # n0 = sqrt(nsq): both sqrts write one n0 tile from one engine
            n0 = smalls.tile([P, OWN_T], F32, tag="n0")
            nc.scalar.activation(out=n0[:, ACT_P1_J:], in_=yv, func=AF.Sqrt,
                                 scale=float(D))
            nc.scalar.activation(out=n0[:, 0:ACT_P1_J], in_=nsq_a,
                                 func=AF.Sqrt)
            r = smalls.tile([P, OWN_T], F32, tag="r")
            nc.vector.reciprocal(r, n0)

            g_ps = [psum_g.tile([P, DA], F32, name=f"g{m}", tag=f"g{m}")
                    for m in range(2)]
            for j in range(OWN_T):
                xh = xh_pool.tile([P, D], F32, tag="xh")
                if j < GPS_P2_J:
                    # ACT path: xh = Copy(x * r) — keeps the Pool queue
                    # trigger-only so a collective trigger never queues
                    # behind compute
                    nc.scalar.activation(out=xh, in_=it[:, j, 0:D],
                                         func=AF.Copy, scale=r[:, j:j + 1])
                else:
                    nc.vector.tensor_scalar(
                        out=xh, in0=it[:, j, 0:D], scalar1=r[:, j:j + 1],
                        scalar2=None, op0=mybir.AluOpType.mult,
                    )
                for m in range(2):
                    nc.tensor.matmul(
                        g_ps[m], lhsT=xh[:, m * P:(m + 1) * P], rhs=it[:, j, :],
                        start=(j == 0), stop=(j == OWN_T - 1),
                    )

            # own-block PE-transpose to xT (lhsT for phase 2), after the
            # G' matmuls so PE starts G' as soon as the first tiles land
            xT = [bigs.tile([P, OWN], F32, name=f"xT{dt}", tag=f"xT{dt}")
                  for dt in range(2)]
            for dt in range(2):
                for g in range(2):          # 4 transposes per PSUM bank
                    pst = psum_tr.tile([P, 4 * P], F32, tag="tr")
                    for jj in range(4):
                        j = g * 4 + jj
                        nc.tensor.transpose(
                            pst[:, jj * P:(jj + 1) * P],
                            it[:, j, dt * P:(dt + 1) * P], ident,
                        )
                    nc.scalar.copy(out=xT[dt][:, g * 4 * P:(g + 1) * 4 * P],
                                   in_=pst)

            # ship the G' partial: PSUM -> SBUF -> DRAM bounce -> AllGather
            # (back-to-back 8-core AllGathers pipeline to ~zero marginal
            # cost on this HW, while AllReduce costs ~17.5 us/iter flat —
            # measured with cc_bench.py)
            g_sb = bigs.tile([P, 2 * DA], F32, tag="g_sb")
            for m in range(2):
                nc.scalar.copy(out=g_sb[:, m * DA:(m + 1) * DA], in_=g_ps[m])
            gin = dram.tile([P, 2 * DA], F32, tag="gin")
            nc.scalar.dma_start(out=gin, in_=g_sb)
            if CC_KIND == "AG":
                # Shared scratchpad output: the AllGather writes ONE copy
                # for the whole chip instead of one per core
                gout = dram.tile([NCORES, P, 2 * DA], F32, tag="gout",
                                 addr_space="Shared")
                nc.gpsimd.collective_compute(
                    "AllGather", mybir.AluOpType.bypass, replica_groups=RG,
                    ins=[gin.opt()], outs=[gout.opt()],
                )
            elif CC_KIND == "RSAG":
                # manual AllReduce: ReduceScatter to shards, AllGather the
                # shards back — ends with the summed G' contiguous in gout
                rs_out = dram.tile([P // NCORES, 2 * DA], F32, tag="rs_out")
                gout = dram.tile([P, 2 * DA], F32, tag="gout")
                nc.gpsimd.collective_compute(
                    "ReduceScatter", mybir.AluOpType.add, replica_groups=RG,
                    ins=[gin.opt()], outs=[rs_out.opt()],
                )
                nc.gpsimd.collective_compute(
                    "AllGather", mybir.AluOpType.bypass, replica_groups=RG,
                    ins=[rs_out.opt()], outs=[gout.opt()],
                )
            else:
                # AllReduce: pricier on the collective device but the
                # readback is 263 KB instead of 2.1 MB — kernel DMA traffic
                # inflates the in-flight collective (measured: AG+4MB DMA
                # = 27 us/iter vs 12 standalone), so fewer bytes win
                gout = dram.tile([P, 2 * DA], F32, tag="gout")
                nc.gpsimd.collective_compute(
                    "AllReduce", mybir.AluOpType.add, replica_groups=RG,
                    ins=[gin.opt()], outs=[gout.opt()],
                )
            state[k] = (it, xT, gout)

        def phase2(k):
            it, xT, gout = state.pop(k)
            if CC_KIND == "AG":
                gath = p2tmp.tile([P, NCORES, 2 * DA], F32, tag="gath")
                nc.sync.dma_start(out=gath,
                                  in_=gout[:].rearrange("k p f -> p k f"))
                # tree-sum the 8 slabs, split DVE / GpSimd so neither
                # engine eats the whole 7-add chain
                s4 = p2tmp.tile([P, 4, 2 * DA], F32, tag="s4")
                nc.vector.tensor_add(s4[:, 0:2], gath[:, 0:2], gath[:, 4:6])
                nc.gpsimd.tensor_add(s4[:, 2:4], gath[:, 2:4], gath[:, 6:8])
                s2 = p2tmp.tile([P, 2, 2 * DA], F32, tag="s2")
                nc.vector.tensor_add(s2[:, 0:1], s4[:, 0:1], s4[:, 2:3])
                nc.gpsimd.tensor_add(s2[:, 1:2], s4[:, 1:2], s4[:, 3:4])
                gg = p2tmp.tile([P, 2 * DA], F32, tag="gg")
                nc.vector.tensor_add(gg, s2[:, 0], s2[:, 1])
            else:
                gg = p2tmp.tile([P, 2 * DA], F32, tag="gg")
                nc.sync.dma_start(out=gg, in_=gout)

            # own rows x G', then divide by the row sum
            outsb = p2tmp.tile([P, OWN_T, D], F32, tag="outsb")
            for j in range(OWN_T):
                oa = psum_o.tile([P, DA], F32, tag="oa")
                for kk in range(2):
                    nc.tensor.matmul(
                        oa, lhsT=xT[kk][:, j * P:(j + 1) * P],
                        rhs=gg[:, kk * DA:(kk + 1) * DA],
                        start=(kk == 0), stop=(kk == 1),
                    )
                rcp = smalls.tile([P, 1], F32, tag="rcp")
                nc.vector.reciprocal(rcp, oa[:, D:DA])
                nc.vector.tensor_scalar_mul(outsb[:, j, :], oa[:, 0:D], rcp)
            # stores ride the SP ring with the readback: both are
            # post-collective, so neither can stall a pre-collective DMA
            nc.sync.dma_start(out=ov[:, 0:OWN_T // 2, :],
                              in_=outsb[:, 0:OWN_T // 2, :])
            nc.sync.dma_start(out=ov[:, OWN_T // 2:, :],
                              in_=outsb[:, OWN_T // 2:, :])

        for k in range(min(DEPTH, iters)):
            phase1(k)
        for k in range(iters):
            if k + DEPTH < iters:
                phase1(k + DEPTH)
            phase2(k)
    return nc


def _get_nc(iters: int = 1):
    if iters not in _nc_cache:
        nc = _build_nc(iters)
        orig = nc.to_json_bytes
        nc.to_json_bytes = lambda: _legalize_sync_waits(orig())
        _nc_cache[iters] = nc
    return _nc_cache[iters]


LAST_RESULTS = None  # BassKernelResults of the most recent run (for profiling)


def kernel(tensor: np.ndarray, trace: bool = False, **trace_kwargs) -> np.ndarray:
    x = np.ascontiguousarray(np.asarray(tensor, dtype=np.float32))
    assert x.shape == (N, D)
    nc = _get_nc()
    in_maps = [
        {"x_own": np.ascontiguousarray(x[i * OWN:(i + 1) * OWN])}
        for i in range(NCORES)
    ]
    global LAST_RESULTS
    LAST_RESULTS = run_bass_kernel_spmd(
        nc, in_maps, core_ids=list(range(NCORES)), trace=trace, **trace_kwargs
    )
    return np.concatenate([r["out"] for r in LAST_RESULTS.results], axis=0)


# revision 33
# speedup vs baseline: 1.3861x; 1.2984x over previous
"""Trainium2 Bass kernel: cosine-similarity message passing (GNN aggregate).

Math (collapsed — the [N,N] similarity matrix is never materialized):
    x_hat = x / max(||x||, eps)                      row-normalized features
    G'    = x_hat.T @ [x | 1]        [D, D+1]        Gram + column-sum s
    oa    = x @ G'                   [N, D+1]        (query-side normalization
                                                      cancels in the ratio)
    out   = oa[:, :D] / oa[:, D:D+1]

Sharding (v2): G' is a sum over ALL rows, so each core computes a partial
G' from only its OWN 1/8 row block (1 MB read instead of the baseline's
redundant 8 MB full-x stream), and an 8-core AllReduce of the [D, D+1]
partials (263 KB fp32) combines them. Phase 2 (own rows x G', divide by
row sum) is unchanged. Per-core HBM traffic drops 10 MB -> ~2.5 MB; the
collective (~13 us) dominates the steady-state per-iteration time.

Environment quirks encoded here:
 - this walrus build accepts at most ONE sync wait per instruction:
   _legalize_sync_waits hoists extras onto same-engine Drain carriers
   (wired via nc.to_json_bytes); tensor_tensor_reduce doesn't compile.
 - the ACT sqrt spline is near-exact on this HW (a Newton refinement
   measurably HURT accuracy in the baseline session).
 - eps in max(||x||, eps) never binds for gaussian rows (min norm ~14).
"""

import numpy as np
from contextlib import ExitStack

import concourse.bass as bass
import concourse.tile as tile
from concourse import mybir
from concourse.masks import make_identity
from concourse.bass_utils import run_bass_kernel_spmd

N, D = 8192, 256
NCORES = 8
P = 128
OWN = N // NCORES            # 1024 rows per core
OWN_T = OWN // P             # 8 own tiles
DA = D + 1                   # 257: x columns + ones column
F32 = mybir.dt.float32
DTR = mybir.dt.float32r
BF16 = mybir.dt.bfloat16
AF = mybir.ActivationFunctionType

# engine split knobs: tiles j < ACT_P1_J do square+reduce on ScalarE, the
# rest on VectorE; tiles j < GPS_P2_J scale on GpSimd, the rest on VectorE.
ACT_P1_J = 4
GPS_P2_J = 4
CC_KIND = "RSAG"               # "AG" (gather + local tree-sum) or "AR"

_nc_cache = {}


def _legalize_sync_waits(bir_bytes: bytes) -> bytes:
    """This walrus build accepts at most ONE sync wait per instruction.
    Tile emits several; hoist the extras onto same-engine Drain
    instructions placed immediately before (queue order preserves the
    semantics of inline waits)."""
    import orjson
    bir = orjson.loads(bir_bytes)
    ctr = [0]

    def fix_block(blk):
        new_list = []
        for ins in blk.get("instructions", []):
            si = ins.get("sync_info")
            if si:
                waits = si.get("on_wait") or []
                if len(waits) > 1:
                    for w in waits[:-1]:
                        ctr[0] += 1
                        new_list.append({
                            "debug": ins.get("debug", 0),
                            "engine": ins["engine"],
                            "ins": [], "outs": [],
                            "name": f"I-lw{ctr[0]}",
                            "opcode": "Drain",
                            "sync_info": {"on_update": [], "on_wait": [w]},
                        })
                    si["on_wait"] = waits[-1:]
            new_list.append(ins)
        blk["instructions"] = new_list
        for sb in blk.get("blocks", []):
            fix_block(sb)

    for f in bir["functions"]:
        for blk in f["blocks"]:
            fix_block(blk)
    return orjson.dumps(bir)


def _build_nc(iters: int = 1):
    nc = bass.Bass(
        "TRN2", target_bir_lowering=False, debug=False, enable_asserts=True,
        num_devices=NCORES,
    )
    x_own = nc.declare_dram_parameter("x_own", [OWN, D], F32, isOutput=False)
    out = nc.declare_dram_parameter("out", [OWN, D], F32, isOutput=True)

    # row = p*OWN_T + t  -> contiguous 8 KB HBM reads per partition
    xo = x_own.ap().rearrange("(p t) d -> p t d", p=P)
    ov = out.ap().rearrange("(p t) d -> p t d", p=P)

    RG = [list(range(NCORES))]

    with tile.TileContext(nc) as tc, ExitStack() as ctx:
        singles = ctx.enter_context(tc.tile_pool(name="singles", bufs=1))
        xh_pool = ctx.enter_context(tc.tile_pool(name="xh", bufs=16))
        trash_pool = ctx.enter_context(tc.tile_pool(name="tra", bufs=1))
        smalls = ctx.enter_context(tc.tile_pool(name="sm", bufs=DEPTH + 1))
        bigs = ctx.enter_context(tc.tile_pool(name="big", bufs=DEPTH + 1))
        p2tmp = ctx.enter_context(tc.tile_pool(name="p2t", bufs=2))
        dram = ctx.enter_context(
            tc.tile_pool(name="dram", bufs=DEPTH + 1, space="DRAM"))
        psum_g = ctx.enter_context(
            tc.tile_pool(name="psg", bufs=1 if TRI else 2, space="PSUM"))
        psum_tr = ctx.enter_context(tc.tile_pool(name="pst", bufs=2, space="PSUM"))
        psum_o = ctx.enter_context(tc.tile_pool(name="pso", bufs=2, space="PSUM"))
        psum_un = (ctx.enter_context(
            tc.tile_pool(name="psu", bufs=2, space="PSUM")) if TRI else None)

        NBUF = DEPTH + 1
        it_bufs = [singles.tile([P, OWN_T, DA], F32, name=f"inbuf{b}")
                   for b in range(NBUF)]
        for b in range(NBUF):
            # ones column, written once; loads only touch [:, :, 0:D]
            nc.gpsimd.memset(it_bufs[b][:, :, D], 1.0)
        ident = singles.tile([P, P], F32)
        make_identity(nc, ident)

        # per-iteration state carried from phase 1 to phase 2
        state = {}

        def phase1(k):
            it = it_bufs[k % NBUF]
            # own-block load, split so phase 1 starts after the first half.
            # Queue discipline: pre-collective DMAs (loads, gin) ride the
            # ACT HWDGE ring; post-collective DMAs (readback, out) ride
            # the SP ring — a ring is FIFO, so a DMA waiting on the
            # collective must never sit ahead of a later phase-1 load.
            nq = 4 if SP_DMA else 2
            for q in range(nq):
                a, b = q * OWN_T // nq, (q + 1) * OWN_T // nq
                nc.scalar.dma_start(out=it[:, a:b, 0:D], in_=xo[:, a:b, :],
                                    single_packet=bool(SP_DMA))

            # nsq split per engine: ACT squares-with-accum for the front
            # tiles, DVE bn_stats for the rest (one tile from both engines
            # would add cross-engine WAW waits)
            nsq_a = smalls.tile([P, ACT_P1_J], F32, tag="nsq_a")
            stats = smalls.tile([P, OWN_T - ACT_P1_J, 6], F32, tag="stats")
            mv = smalls.tile([P, OWN_T - ACT_P1_J, 2], F32, tag="mv")
            for j in range(OWN_T):
                if j < ACT_P1_J:
                    # dedicated slot per op: a reused slot would add a WAW
                    # semaphore and Activation allows only one wait
                    tr = trash_pool.tile([P, D], F32, tag=f"ta{j}")
                    nc.scalar.activation(
                        out=tr, in_=it[:, j, 0:D], func=AF.Square,
                        accum_out=nsq_a[:, j:j + 1],
                    )
                else:
                    jj = j - ACT_P1_J
                    nc.vector.bn_stats(out=stats[:, jj, :], in_=it[:, j, 0:D])
                    nc.vector.bn_aggr(out=mv[:, jj, :], in_=stats[:, jj, :])
            # nsq_v = D*(var + mean^2); mean^2 << var for gaussian rows
            ymm = smalls.tile([P, OWN_T - ACT_P1_J], F32, tag="ymm")
            nc.vector.tensor_mul(ymm, mv[:, :, 0], mv[:, :, 0])
            yv = smalls.tile([P, OWN_T - ACT_P1_J], F32, tag="yv")
            nc.vector.tensor_add(yv, ymm, mv[:, :, 1])

            # n0 = sqrt(nsq): both sqrts write one n0 tile from one engine
            n0 = smalls.tile([P, OWN_T], F32, tag="n0")
            nc.scalar.activation(out=n0[:, ACT_P1_J:], in_=yv, func=AF.Sqrt,
                                 scale=float(D))
            nc.scalar.activation(out=n0[:, 0:ACT_P1_J], in_=nsq_a,
                                 func=AF.Sqrt)
            r = smalls.tile([P, OWN_T], F32, tag="r")
            nc.vector.reciprocal(r, n0)

            if USE_F32R:
                # numerator G_x blocks accumulate at f32r rate; the ones
                # column (denominator) gets its own exact-fp32 chain
                g_ps = [psum_g.tile([P, D], F32, name="g0", tag="g0"),
                        psum_g.tile([P, D], F32, name="g1", tag="g1")]
                g_ones = [psum_g.tile([P, 1], F32, name="gons0", tag="gons0"),
                          psum_g.tile([P, 1], F32, name="gons1", tag="gons1")]
                m1w = DA
                xr_bufs, ir_bufs = [], []
            else:
                m1w = P + 1 if TRI else DA
                g_ps = [psum_g.tile([P, DA], F32, name="g0", tag="g0"),
                        psum_g.tile([P, m1w], F32, name="g1", tag="g1")]
            for j in range(OWN_T):
                xh = xh_pool.tile([P, D], F32, tag="xh")
                if j < GPS_P2_J and XH_ENG == "act":
                    nc.scalar.activation(out=xh, in_=it[:, j, 0:D],
                                         func=AF.Copy, scale=r[:, j:j + 1])
                elif j < GPS_P2_J:
                    nc.gpsimd.tensor_scalar(
                        out=xh, in0=it[:, j, 0:D], scalar1=r[:, j:j + 1],
                        scalar2=None, op0=mybir.AluOpType.mult,
                    )
                else:
                    nc.vector.tensor_scalar(
                        out=xh, in0=it[:, j, 0:D], scalar1=r[:, j:j + 1],
                        scalar2=None, op0=mybir.AluOpType.mult,
                    )
                if USE_F32R:
                    # f32r-certified copies of the numerator operands (the
                    # denominator chains below keep the exact-f32 originals)
                    xh_r = xh_pool.tile([P, D], DTR, tag="xh_r")
                    it_r = xh_pool.tile([P, D], DTR, tag="it_r")
                    if j % 2 == 0:
                        nc.scalar.activation(out=xh_r, in_=xh, func=AF.Copy)
                        nc.vector.tensor_scalar(
                            out=it_r, in0=it[:, j, 0:D], scalar1=1.0,
                            scalar2=None, op0=mybir.AluOpType.mult)
                    else:
                        nc.vector.tensor_scalar(
                            out=xh_r, in0=xh, scalar1=1.0,
                            scalar2=None, op0=mybir.AluOpType.mult)
                        nc.scalar.activation(out=it_r, in_=it[:, j, 0:D],
                                             func=AF.Copy)
                if USE_F32R:
                    ss = dict(start=(j == 0), stop=(j == OWN_T - 1))
                    for m in range(2):
                        nc.tensor.matmul(
                            g_ones[m],
                            lhsT=xh[:, m * P:(m + 1) * P],
                            rhs=it[:, j, D:DA], **ss,
                        )
                    xr_bufs.append((xh_r, it_r))
                else:
                    nc.tensor.matmul(
                        g_ps[0], lhsT=xh[:, 0:P], rhs=it[:, j, :],
                        start=(j == 0), stop=(j == OWN_T - 1),
                    )
                    nc.tensor.matmul(
                        g_ps[1], lhsT=xh[:, P:2 * P],
                        rhs=it[:, j, (DA - m1w):DA],
                        start=(j == 0), stop=(j == OWN_T - 1),
                    )

            if USE_F32R:
                # each f32r chain runs with no other f32r chain interleaved
                for m in range(2):
                    for j, (xh_r, it_r) in enumerate(xr_bufs):
                        nc.tensor.matmul(
                            g_ps[m], lhsT=xh_r[:, m * P:(m + 1) * P],
                            rhs=it_r,
                            start=(j == 0), stop=(j == OWN_T - 1),
                        )

            # own-block PE-transpose to xT (lhsT for phase 2), after the
            # G' matmuls so PE starts G' as soon as the first tiles land
            xT = [bigs.tile([P, OWN], F32, name=f"xT{dt}", tag=f"xT{dt}")
                  for dt in range(2)]
            for dt in range(2):
                for g in range(2):          # 4 transposes per PSUM bank
                    pst = psum_tr.tile([P, 4 * P], F32, tag="tr")
                    for jj in range(4):
                        j = g * 4 + jj
                        nc.tensor.transpose(
                            pst[:, jj * P:(jj + 1) * P],
                            it[:, j, dt * P:(dt + 1) * P], ident,
                        )
                    nc.scalar.copy(out=xT[dt][:, g * 4 * P:(g + 1) * 4 * P],
                                   in_=pst)

            # ship the G' partial: PSUM -> SBUF -> DRAM bounce -> AllGather
            # (back-to-back 8-core AllGathers pipeline to ~zero marginal
            # cost on this HW, while AllReduce costs ~17.5 us/iter flat —
            # measured with cc_bench.py)
            if USE_F32R:
                gw = 2 * DA              # [Gx0 | ones0 | Gx1 | ones1]
                g_sb = bigs.tile([P, gw], F32, tag="g_sb")
                nc.scalar.copy(out=g_sb[:, 0:D], in_=g_ps[0])
                nc.scalar.copy(out=g_sb[:, D:DA], in_=g_ones[0])
                nc.scalar.copy(out=g_sb[:, DA:DA + D], in_=g_ps[1])
                nc.scalar.copy(out=g_sb[:, DA + D:gw], in_=g_ones[1])
            else:
                gw = DA + m1w            # 386 block-tri, 514 full
                g_sb = bigs.tile([P, gw], F32, tag="g_sb")
                nc.scalar.copy(out=g_sb[:, 0:DA], in_=g_ps[0])
                nc.scalar.copy(out=g_sb[:, DA:gw], in_=g_ps[1])
            gin = dram.tile([P, gw], F32, tag="gin")
            nc.scalar.dma_start(out=gin, in_=g_sb)
            if CC_KIND == "AG":
                # Shared scratchpad output: the AllGather writes ONE copy
                # for the whole chip instead of one per core
                gout = dram.tile([NCORES, P, 2 * DA], F32, tag="gout",
                                 addr_space="Shared")
                nc.gpsimd.collective_compute(
                    "AllGather", mybir.AluOpType.bypass, replica_groups=RG,
                    ins=[gin.opt()], outs=[gout.opt()],
                )
            elif CC_KIND == "RSAG":
                # manual AllReduce: ReduceScatter to shards, AllGather the
                # shards back — ends with the summed G' contiguous in gout
                rs_out = dram.tile([P // NCORES, 2 * DA], F32, tag="rs_out")
                gout = dram.tile([P, 2 * DA], F32, tag="gout")
                nc.gpsimd.collective_compute(
                    "ReduceScatter", mybir.AluOpType.add, replica_groups=RG,
                    ins=[gin.opt()], outs=[rs_out.opt()],
                )
                nc.gpsimd.collective_compute(
                    "AllGather", mybir.AluOpType.bypass, replica_groups=RG,
                    ins=[rs_out.opt()], outs=[gout.opt()],
                )
            elif CC_KIND == "NONE":
                # timing diagnostic only: no cross-core combine (WRONG
                # results) — measures the kernel's non-collective floor
                gout = dram.tile([P, gw], F32, tag="gout")
                nc.sync.dma_start(out=gout, in_=gin)
            else:
                # AllReduce: pricier on the collective device but the
                # readback is 263 KB instead of 2.1 MB — kernel DMA traffic
                # inflates the in-flight collective (measured: AG+4MB DMA
                # = 27 us/iter vs 12 standalone), so fewer bytes win
                gout = dram.tile([P, gw], F32, tag="gout",
                                 addr_space="Shared" if SHOUT else "Local")
                nc.gpsimd.collective_compute(
                    "AllReduce", mybir.AluOpType.add, replica_groups=RG,
                    ins=[gin.opt()], outs=[gout.opt()],
                )
            state[k] = (it, xT, gout)

        def phase2(k):
            it, xT, gout = state.pop(k)
            if CC_KIND == "AG":
                gath = p2tmp.tile([P, NCORES, 2 * DA], F32, tag="gath")
                nc.sync.dma_start(out=gath,
                                  in_=gout[:].rearrange("k p f -> p k f"))
                # tree-sum the 8 slabs, split DVE / GpSimd so neither
                # engine eats the whole 7-add chain
                s4 = p2tmp.tile([P, 4, 2 * DA], F32, tag="s4")
                nc.vector.tensor_add(s4[:, 0:2], gath[:, 0:2], gath[:, 4:6])
                nc.gpsimd.tensor_add(s4[:, 2:4], gath[:, 2:4], gath[:, 6:8])
                s2 = p2tmp.tile([P, 2, 2 * DA], F32, tag="s2")
                nc.vector.tensor_add(s2[:, 0:1], s4[:, 0:1], s4[:, 2:3])
                nc.gpsimd.tensor_add(s2[:, 1:2], s4[:, 1:2], s4[:, 3:4])
                gg = p2tmp.tile([P, 2 * DA], F32, tag="gg")
                nc.vector.tensor_add(gg, s2[:, 0], s2[:, 1])
            else:
                gw = (2 * DA if USE_F32R else DA + (P + 1 if TRI else DA))
                gg = p2tmp.tile([P, gw], F32, tag="gg")
                nc.sync.dma_start(out=gg, in_=gout)

            if TRI and not USE_F32R:
                # rebuild G' rows 128:256: block10 = block01.T (one PE
                # transpose), then block11+ones from the packed tail
                g1f = p2tmp.tile([P, DA], F32, tag="g1f")
                ptr = psum_un.tile([P, P], F32, tag="untr")
                nc.tensor.transpose(ptr, gg[:, P:2 * P], ident)
                nc.scalar.copy(out=g1f[:, 0:P], in_=ptr)
                nc.scalar.copy(out=g1f[:, P:DA], in_=gg[:, DA:])
                rhs_k = [gg[:, 0:DA], g1f[:, :]]
            else:
                rhs_k = [gg[:, 0:DA], gg[:, DA:2 * DA]]

            # own rows x G', then divide by the row sum
            outsb = p2tmp.tile([P, OWN_T, D], F32, tag="outsb")
            for j in range(OWN_T):
                oa = psum_o.tile([P, DA], F32, tag="oa")
                for kk in range(2):
                    nc.tensor.matmul(
                        oa, lhsT=xT[kk][:, j * P:(j + 1) * P],
                        rhs=rhs_k[kk],
                        start=(kk == 0), stop=(kk == 1),
                    )
                rcp = smalls.tile([P, 1], F32, tag="rcp")
                nc.vector.reciprocal(rcp, oa[:, D:DA])
                nc.vector.tensor_scalar_mul(outsb[:, j, :], oa[:, 0:D], rcp)
            # stores ride the SP ring with the readback: both are
            # post-collective, so neither can stall a pre-collective DMA
            nq = 4 if SP_DMA else 2
            for q in range(nq):
                a, b = q * OWN_T // nq, (q + 1) * OWN_T // nq
                nc.sync.dma_start(out=ov[:, a:b, :], in_=outsb[:, a:b, :],
                                  single_packet=bool(SP_DMA))

        for k in range(min(DEPTH, iters)):
            phase1(k)
        for k in range(iters):
            if k + DEPTH < iters:
                phase1(k + DEPTH)
            phase2(k)
    return nc


def _get_nc(iters: int = 1):
    key = (iters, CC_KIND, XH_ENG, DEPTH, GPS_P2_J, SP_DMA, TRI, SHOUT, USE_F32R)
    if key not in _nc_cache:
        nc = _build_nc(iters)
        orig = nc.to_json_bytes
        nc.to_json_bytes = lambda: _legalize_sync_waits(orig())
        _nc_cache[key] = nc
    return _nc_cache[key]


LAST_RESULTS = None  # BassKernelResults of the most recent run (for profiling)


def kernel(tensor: np.ndarray, trace: bool = False, **trace_kwargs) -> np.ndarray:
    x = np.ascontiguousarray(np.asarray(tensor, dtype=np.float32))
    assert x.shape == (N, D)
    nc = _get_nc()
    in_maps = [
        {"x_own": np.ascontiguousarray(x[i * OWN:(i + 1) * OWN])}
        for i in range(NCORES)
    ]
    global LAST_RESULTS
    LAST_RESULTS = run_bass_kernel_spmd(
        nc, in_maps, core_ids=list(range(NCORES)), trace=trace, **trace_kwargs
    )
    return np.concatenate([r["out"] for r in LAST_RESULTS.results], axis=0)
